# revision 1
# baseline (speedup 1.0000x reference)
"""Causal self-attention Trainium2 kernel, v2.

Sharding: 8 cores = 4 batches x 2 head-groups (8 heads each).

Per-core dataflow:
  - QKV projections as fp8e4 DoubleRow matmuls (256-feature contraction
    per instruction) with a hi/lo split of both x and W (host-prepared):
    q = xh@Wh + xl@Wh + xh@Wl  -- 3 DoubleRow passes = 6N cycles vs
    fp32r's 8N, with ~0.1% error.
  - q,k stored bf16 (rate-1 matmuls at any N, so causal diagonal blocks
    need no 256-col widening); scores per k-tile into PSUM.
  - exp on ACT writes P^T directly as bf16; causal triangle zeroed on
    GPSIMD (affine_select); PV matmuls in bf16 with a ones-column in the
    V tile accumulating softmax denominators.
  - out = PV / rowsum via DVE reciprocal + GPSIMD partition broadcast.
  - y = outT.T @ Wo in fp32r (partial; host sums the 2 head-groups).

Scheduling: projection chains for chunk ch+1 and output-projection tiles
for chunk ch-1 are interleaved between attention k-tiles of chunk ch so
the PE never idles while ACT paces the softmax.
"""
import numpy as np

B, T, D, H = 4, 2048, 1024, 16
HD = D // H            # 64
NCORES = 8
HPC = 8                # heads per core
FPC = HPC * HD         # 512 feature cols per core
NPAIR = HPC // 2       # 4 head pairs
NG = 4                 # fp8 DoubleRow contraction groups (256 feats each)
KT = T // 128          # 16 k-tiles
NCH = T // 512         # 4 q-chunks
WSC = 50.0             # host weight scaling before fp8 quantization
QKS = 16.0             # q/k fp8 storage scale
VSTR = 65              # per-k-tile stride in vaug free dim
HSTR = KT * VSTR       # per-head stride in vaug free dim

_CACHE = {}


def _build():
    import concourse.mybir as mybir
    import concourse.tile as tile
    from concourse import bacc
    from contextlib import ExitStack

    f32 = mybir.dt.float32
    f32r = mybir.dt.float32r
    bf16 = mybir.dt.bfloat16
    f8 = mybir.dt.float8e4
    DR = mybir.MatmulPerfMode.DoubleRow
    Exp = mybir.ActivationFunctionType.Exp
    Alu = mybir.AluOpType

    nc = bacc.Bacc("TRN2", target_bir_lowering=False, debug=False,
                   num_devices=NCORES)
    # x hi/lo fp8, repacked host-side as [ch, 128, g, i, tok]:
    # feature = 256*g + 128*i + partition
    xh = nc.dram_tensor("xh", [NCH, 128, NG * 1024], f8, kind="ExternalInput")
    xl = nc.dram_tensor("xl", [NCH, 128, NG * 1024], f8, kind="ExternalInput")
    # fp8 weights, 6 tensors: q/k: [p][g][i][f128]; v: [g][i][f512]
    Wsec = [nc.dram_tensor(nm, [128, 4096], f8, kind="ExternalInput")
            for nm in ("Wqh", "Wql", "Wkh", "Wkl", "Wvh", "Wvl")]
    BQ = nc.dram_tensor("BQ", [128, NPAIR], f32, kind="ExternalInput")
    BK = nc.dram_tensor("BK", [128, NPAIR], f32, kind="ExternalInput")
    BV = nc.dram_tensor("BV", [FPC], f32, kind="ExternalInput")
    Wo = nc.dram_tensor("Wo", [FPC, D], f32r, kind="ExternalInput")
    y = nc.dram_tensor("y", [T, D], f32, kind="ExternalOutput")

    with tile.TileContext(nc) as tc, ExitStack() as es:
        pers = es.enter_context(tc.tile_pool(name="pers", bufs=1))
        xsp = es.enter_context(tc.tile_pool(name="xsp", bufs=3))
        ptp = es.enter_context(tc.tile_pool(name="ptp", bufs=10))
        nrm = es.enter_context(tc.tile_pool(name="nrm", bufs=2))
        obp = es.enter_context(tc.tile_pool(name="obp", bufs=5))
        stgp = es.enter_context(tc.tile_pool(name="stgp", bufs=4))
        psA = es.enter_context(tc.tile_pool(name="psA", bufs=2, space="PSUM"))
        psS = es.enter_context(tc.tile_pool(name="psS", bufs=2, space="PSUM"))
        psO = es.enter_context(tc.tile_pool(name="psO", bufs=1, space="PSUM"))

        ws_sb = [pers.tile([128, 4096], f8, tag=f"ws{i}", name=f"ws{i}")
                 for i in range(6)]
        bq_sb = pers.tile([128, NPAIR], f32, tag="bq")
        bk_sb = pers.tile([128, NPAIR], f32, tag="bk")
        bv_row = pers.tile([1, FPC], f32, tag="bvr")
        bv_bc = pers.tile([128, FPC], f32, tag="bvb")
        wo_sb = pers.tile([128, NPAIR * D], f32r, tag="wo")
        vaug = pers.tile([128, HPC * HSTR], bf16, tag="vaug")
        # q/k in fp8 for DoubleRow scores: tile u holds pairs (2u, 2u+1);
        # partition = 64*(pr%2) + 32*head + hd%32, free = (hd-half j,
        # q-or-k, token) -- one tile so each remap DMA moves q and k
        qkT8 = [pers.tile([128, 2, 2, T], f8, tag=f"qkT8{u}",
                          name=f"qkT8{u}") for u in range(2)]
        outT = [pers.tile([128, T], f32r, tag=f"oT{p}", name=f"oT{p}")
                for p in range(NPAIR)]

        vaug4 = vaug[:].rearrange("p (h k x) -> p h k x", h=HPC, k=KT)

        xtiles = {}

        def emit_xdma(ch):
            xh_sb = xsp.tile([128, NG * 1024], f8, tag="xh", name="xh_sb")
            xl_sb = xsp.tile([128, NG * 1024], f8, tag="xl", name="xl_sb")
            if ch == 0:
                half = NG * 512
                nc.sync.dma_start(xh_sb[:, 0:half], xh[ch][:, 0:half])
                nc.sync.dma_start(xl_sb[:, 0:half], xl[ch][:, 0:half])
                nc.sync.dma_start(xh_sb[:, half:], xh[ch][:, half:])
                nc.sync.dma_start(xl_sb[:, half:], xl[ch][:, half:])
            else:
                nc.sync.dma_start(xh_sb[:], xh[ch])
                nc.sync.dma_start(xl_sb[:], xl[ch])
            xtiles[ch] = [xx[:, g * 1024:(g + 1) * 1024].rearrange(
                "p (i t) -> p i t", i=2)
                for xx in (xh_sb, xl_sb) for g in range(NG)]

        def w_ap(sec, p, g):
            base = p * 1024 + g * 256
            return ws_sb[sec][:, base:base + 256].rearrange(
                "p (i f) -> p i f", i=2)

        def wv_ap(sec, g):
            base = g * 1024
            return ws_sb[sec][:, base:base + 1024].rearrange(
                "p (i f) -> p i f", i=2)

        # (x-part, w-section) term order: xl last so its DMA can trail
        # at startup. psum partitions are ordered (hd-half j, head, hd%32)
        # by the host weight packing; both evacs write scaled fp8 into one
        # staging tile whose two j-halves are then DMA'd into the
        # partition-sliced qkT8 layout (one DMA moves q and k together).
        def proj_qk(ch, p):
            xg = xtiles[ch]
            stg = stgp.tile([128, 2, 512], f8, tag="stg", name="stg")
            for qk, (sec, bsb) in enumerate(((0, bq_sb), (2, bk_sb))):
                ps = psA.tile([128, 512], f32, tag="psA", name="psqk")
                n = 0
                for (xt, ws) in ((0, sec), (0, sec + 1), (1, sec)):
                    for g in range(NG):
                        nc.tensor.matmul(
                            ps[:], w_ap(ws, p, g), xg[xt * NG + g],
                            start=(n == 0), stop=(n == 3 * NG - 1),
                            perf_mode=DR)
                        n += 1
                nc.vector.tensor_scalar(
                    stg[:, qk, :], ps[:], QKS / WSC, bsb[:, p:p + 1],
                    Alu.mult, Alu.add)
            u, e = p // 2, p % 2
            for j in range(2):
                nc.sync.dma_start(
                    qkT8[u][64 * e:64 * e + 64, j, :,
                            512 * ch:512 * (ch + 1)],
                    stg[64 * j:64 * j + 64, :, :])

        def proj_v(ch, tj):
            xg = xtiles[ch]
            lo = (tj - 4 * ch) * 128
            ps = psA.tile([128, FPC], f32, tag="psA", name="psv")
            n = 0
            for (xt, ws) in ((0, 4), (0, 5), (1, 4)):
                for g in range(NG):
                    nc.tensor.matmul(
                        ps[:], xg[xt * NG + g][:, :, lo:lo + 128],
                        wv_ap(ws, g),
                        start=(n == 0), stop=(n == 3 * NG - 1),
                        perf_mode=DR)
                    n += 1
            nc.vector.scalar_tensor_tensor(
                vaug4[:, :, tj, 0:64],
                ps[:].rearrange("p (h x) -> p h x", h=HPC),
                1.0 / WSC,
                bv_bc[:].rearrange("p (h x) -> p h x", h=HPC),
                Alu.mult, Alu.add)

        def proj_chunk_fill(ch):
            out = []
            for p in range(NPAIR):
                out.append(lambda p=p: proj_qk(ch, p))
            for tj in range(4 * ch, 4 * (ch + 1)):
                out.append(lambda tj=tj: proj_v(ch, tj))
            return out

        Copy = mybir.ActivationFunctionType.Copy

        def outproj_tile(tj, act_evac=False):
            ob = obp.tile([128, D], f32, tag="ob", name="ob")
            for n in range(2):
                ps = psA.tile([128, 512], f32, tag="psA", name="pspr")
                for p in range(NPAIR):
                    nc.tensor.matmul(
                        ps[:],
                        outT[p][:, 128 * tj:128 * (tj + 1)],
                        wo_sb[:, p * D + 512 * n:p * D + 512 * (n + 1)],
                        start=(p == 0), stop=(p == NPAIR - 1))
                if act_evac:
                    # ACT evac + per-half writeback shortens the final
                    # drain (keeps the tail off the DVE queue too)
                    nc.scalar.activation(ob[:, 512 * n:512 * (n + 1)],
                                         ps[:], Copy)
                    nc.sync.dma_start(
                        y[128 * tj:128 * (tj + 1),
                          512 * n:512 * (n + 1)],
                        ob[:, 512 * n:512 * (n + 1)])
                else:
                    nc.vector.tensor_copy(ob[:, 512 * n:512 * (n + 1)],
                                          ps[:])
            if not act_evac:
                nc.sync.dma_start(y[128 * tj:128 * (tj + 1), :], ob[:])

        def emit_norm(p, ch, po, c0, c1):
            w = c1 - c0
            for h in range(2):
                og = nrm.tile([65, 512], f32, tag="og", name="og")
                nc.vector.tensor_copy(og[:, 0:w], po[h][:, c0:c1])
                rec = nrm.tile([1, 512], f32, tag="rec", name="rec")
                nc.vector.reciprocal(rec[:, 0:w], og[64:65, 0:w])
                bc = nrm.tile([64, 512], f32, tag="bc", name="bc")
                nc.gpsimd.partition_broadcast(bc[:, 0:w], rec[:, 0:w])
                nc.vector.tensor_mul(
                    outT[p][64 * h:64 * (h + 1),
                            512 * ch + c0:512 * ch + c1],
                    og[0:64, 0:w], bc[:, 0:w])

        def attn_column(p, ch, fill, tailjobs=None):
            """fill: iterator of emitters to interleave between k-tiles."""
            po = [psO.tile([65, 512], f32, tag=f"po{h}", name=f"po{h}")
                  for h in range(2)]
            nk = 4 * ch + 4

            def emit_pv(kt, pt):
                qo = max(kt - 4 * ch, 0) * 128
                for h in range(2):
                    nc.tensor.matmul(
                        po[h][:, qo:],
                        vaug4[:, 2 * p + h, kt, :],
                        pt[:, h, qo:],
                        start=(kt == 0), stop=(kt == nk - 1),
                        skip_group_check=True)

            pend = []
            for kt in range(nk):
                dg = kt - 4 * ch
                qo = max(dg, 0) * 128
                pt = ptp.tile([128, 2, 512], bf16, tag="pt", name="pt")
                ss = psS.tile([128, 2, 512], f32, tag="psS", name="ss")
                u, e = p // 2, p % 2
                for h in range(2):
                    b0 = 64 * e + 32 * h
                    nc.tensor.matmul(
                        ss[:, h, qo:],
                        qkT8[u][b0:b0 + 32, :, 1,
                                128 * kt:128 * (kt + 1)],
                        qkT8[u][b0:b0 + 32, :, 0,
                                512 * ch + qo:512 * (ch + 1)],
                        start=True, stop=True, perf_mode=DR,
                        tile_position=(b0, 0))
                nc.scalar.activation(pt[:, :, qo:], ss[:, :, qo:],
                                     Exp, scale=0.125 / (QKS * QKS))
                if dg >= 0:
                    # zero the upper (q < k) triangle of the diagonal block
                    blk = pt[:, :, qo:qo + 128]
                    nc.gpsimd.affine_select(
                        out=blk, in_=blk,
                        compare_op=mybir.AluOpType.is_ge,
                        fill=0.0, base=0,
                        pattern=[[0, 2], [1, 128]],
                        channel_multiplier=-1)
                # fill, then 2-tile-deep software-pipelined PV so the
                # PE never waits inline on an exp
                f = next(fill, None)
                if f:
                    f()
                pend.append((kt, pt))
                if len(pend) > 4:
                    emit_pv(*pend.pop(0))
            if tailjobs is None:
                for pv in pend:
                    f = next(fill, None)
                    if f:
                        f()
                    emit_pv(*pv)
                emit_norm(p, ch, po, 0, 512)
            else:
                # last column: po[:, 0:128*q] is complete as soon as the
                # q-th diagonal PV lands -- normalize and project each
                # quarter as it completes, with reserved chunk-2 output
                # tiles filling each normalize chain's latency
                while len(pend) > 2:
                    emit_pv(*pend.pop(0))
                emit_norm(p, ch, po, 0, 128)
                tailjobs[0]()
                tailjobs[4]()
                emit_norm(p, ch, po, 128, 256)
                tailjobs[1]()
                emit_pv(*pend.pop(0))
                tailjobs[5]()
                emit_pv(*pend.pop(0))
                emit_norm(p, ch, po, 256, 384)
                emit_norm(p, ch, po, 384, 512)
                tailjobs[2]()
                tailjobs[6]()
                tailjobs[3]()
                tailjobs[7]()

        # ---------------- schedule ----------------
        # startup DMA order: q weights and x first (in first-use order,
        # split so the first projection chain can start ~4us in)
        H2 = 2048
        nc.sync.dma_start(ws_sb[0][:, 0:H2], Wsec[0][:, 0:H2])
        emit_xdma(0)
        nc.sync.dma_start(ws_sb[1][:, 0:H2], Wsec[1][:, 0:H2])
        nc.sync.dma_start(bq_sb[:], BQ[:])
        nc.sync.dma_start(ws_sb[2][:, 0:H2], Wsec[2][:, 0:H2])
        nc.sync.dma_start(ws_sb[3][:, 0:H2], Wsec[3][:, 0:H2])
        nc.sync.dma_start(bk_sb[:], BK[:])
        nc.sync.dma_start(bv_row[:], BV[:])
        nc.sync.dma_start(ws_sb[0][:, H2:], Wsec[0][:, H2:])
        nc.sync.dma_start(ws_sb[1][:, H2:], Wsec[1][:, H2:])
        nc.sync.dma_start(ws_sb[2][:, H2:], Wsec[2][:, H2:])
        nc.sync.dma_start(ws_sb[3][:, H2:], Wsec[3][:, H2:])
        nc.gpsimd.partition_broadcast(bv_bc[:], bv_row[:])
        nc.vector.memset(vaug4[:, :, :, 64:65], 1.0)

        fill0 = proj_chunk_fill(0)
        # qk p0-1, V weights, qk p2-3, x prefetch for chunk 1, V chains
        for f in fill0[:2]:
            f()
        nc.sync.dma_start(ws_sb[4][:], Wsec[4][:])
        nc.sync.dma_start(ws_sb[5][:], Wsec[5][:])
        for f in fill0[2:4]:
            f()
        emit_xdma(1)
        for f in fill0[4:]:
            f()

        # ---- deadline-packed fill assignment over the k-tile slots ----
        slots = []           # (ch, p, kt) in emission order
        slot_of = {}
        for ch in range(NCH):
            for p in range(NPAIR):
                for kt in range(4 * ch + 4):
                    slot_of[(ch, p, kt)] = len(slots)
                    slots.append((ch, p, kt))
        nslots = len(slots)
        chunk_start = {ch: slot_of[(ch, 0, 0)] for ch in range(NCH)}
        chunk_end = {ch: slot_of[(ch, NPAIR - 1, 4 * ch + 3)]
                     for ch in range(NCH)}

        items = []  # (deadline, avail, fn)
        for ch in range(1, NCH):
            av = chunk_start[ch - 1]
            for p in range(NPAIR):
                dl = slot_of[(ch, p, 0)] - 4
                items.append((dl, av, lambda ch=ch, p=p: proj_qk(ch, p)))
            for tj in range(4 * ch, 4 * (ch + 1)):
                dl = slot_of[(ch, 0, tj)] - 2
                items.append((dl, av, lambda ch=ch, tj=tj: proj_v(ch, tj)))
        for tj in range(4 * (NCH - 2)):
            items.append((nslots - 1, chunk_end[tj // 4] + 1,
                          lambda tj=tj: outproj_tile(tj)))
        items.append((chunk_end[0] + 8, 0, lambda: nc.sync.dma_start(
            wo_sb[:].rearrange("p (t c) -> p t c", t=NPAIR),
            Wo[:].rearrange("(t p) c -> p t c", p=128))))

        assigned = {}
        for dl, av, fn in sorted(items, key=lambda it: -it[0]):
            s = min(dl, nslots - 1)
            while s >= av and s in assigned:
                s -= 1
            if s < av:
                s = av
                while s in assigned:
                    s += 1
            assigned[s] = fn

        fills = {}
        for s, fn in assigned.items():
            fills.setdefault(s, []).append(fn)

        def _slotfill(seq):
            for fl in seq:
                yield fl[0] if fl else None
            while True:
                yield None

        last = NCH - 1
        for ch in range(NCH):
            if 0 < ch and ch + 1 < NCH:
                emit_xdma(ch + 1)
            for p in range(NPAIR):
                nk = 4 * ch + 4
                seq = []
                for kt in range(nk):
                    fl = fills.get(slot_of[(ch, p, kt)], [])
                    seq.append(fl)
                it = _slotfill(seq)
                tailjobs = None
                if ch == last and p == NPAIR - 1:
                    tailjobs = [
                        lambda tj=tj: outproj_tile(tj, act_evac=True)
                        for tj in list(range(4 * (last - 1), 4 * last)) +
                        list(range(4 * last, 4 * last + 4))]
                attn_column(p, ch, it, tailjobs)
    nc.compile()
    return nc


class _Runner:
    def __init__(self, nc):
        import jax
        from jax.sharding import Mesh, PartitionSpec, NamedSharding
        from jax.experimental.shard_map import shard_map
        import concourse.mybir as mybir
        from concourse.bass2jax import (_bass_exec_p, partition_id_tensor,
                                        install_neuronx_cc_hook)
        install_neuronx_cc_hook()
        self.jax = jax
        part = nc.partition_id_tensor.name if nc.partition_id_tensor else None
        in_names, out_names, out_avals = [], [], []
        for alloc in nc.m.functions[0].allocations:
            if not isinstance(alloc, mybir.MemoryLocationSet):
                continue
            name = alloc.memorylocations[0].name
            if alloc.kind == "ExternalInput":
                if name != part:
                    in_names.append(name)
            elif alloc.kind == "ExternalOutput":
                out_names.append(name)
                out_avals.append(jax.core.ShapedArray(
                    tuple(alloc.tensor_shape), mybir.dt.np(alloc.dtype)))
        self.in_names, self.out_names, self.out_avals = in_names, out_names, out_avals
        all_in = list(in_names) + list(out_names) + ([part] if part else [])

        def _body(*args):
            ops = list(args)
            if part:
                ops.append(partition_id_tensor())
            return tuple(_bass_exec_p.bind(
                *ops, out_avals=tuple(out_avals), in_names=tuple(all_in),
                out_names=tuple(out_names), lowering_input_output_aliases=(),
                sim_require_finite=True, sim_require_nnan=True, nc=nc))

        devices = jax.devices()[:NCORES]
        mesh = Mesh(np.asarray(devices), ("core",))
        nin = len(in_names) + len(out_names)
        self.fn = jax.jit(
            shard_map(_body, mesh=mesh,
                      in_specs=(PartitionSpec("core"),) * nin,
                      out_specs=(PartitionSpec("core"),) * len(out_names),
                      check_rep=False),
            keep_unused=True)
        self.sharding = NamedSharding(mesh, PartitionSpec("core"))

    def put_inputs(self, in_maps):
        args = []
        for name in self.in_names:
            cat = np.concatenate([np.asarray(m[name]) for m in in_maps], axis=0)
            args.append(self.jax.device_put(cat, self.sharding))
        for av in self.out_avals:
            z = np.zeros((NCORES * av.shape[0], *av.shape[1:]), av.dtype)
            args.append(self.jax.device_put(z, self.sharding))
        return args

    def run_np(self, args):
        outs = self.fn(*args)
        return [
            {n: np.asarray(outs[i]).reshape(NCORES, *self.out_avals[i].shape)[c]
             for i, n in enumerate(self.out_names)}
            for c in range(NCORES)
        ]


def _get_runner():
    if "r" not in _CACHE:
        nc = _build()
        _CACHE["nc"] = nc
        _CACHE["r"] = _Runner(nc)
    return _CACHE["r"]


def _rne11(a):
    """Round fp32 to 11 mantissa bits, round-to-nearest-even (= hw fp32r)."""
    ai = np.ascontiguousarray(a, dtype=np.float32).view(np.uint32).astype(np.uint64)
    lsb = (ai >> 12) & 1
    out = (((ai + 2047 + lsb) >> 12) << 12).astype(np.uint32)
    return out.view(np.float32)


def _q8(a):
    import ml_dtypes
    a = np.clip(np.asarray(a, np.float32), -240.0, 240.0)
    return a.astype(ml_dtypes.float8_e4m3)


def _pack_bias(b):
    # [512] -> [128, NPAIR] with the (j, h, s) psum-partition order,
    # pre-scaled by the q/k fp8 storage scale
    br = (b * QKS).reshape(NPAIR, 2, 2, 32).transpose(2, 1, 3, 0)
    return np.ascontiguousarray(br.reshape(128, NPAIR).astype(np.float32))


def make_in_maps(x, Wqkv, bqkv, Wo, bo=None, mask=None):
    # x repack: [ch, part, g, i, tok], feature = 256g + 128i + part
    xhs, xls = [], []
    for b in range(B):
        xr = np.ascontiguousarray(
            x[b].reshape(NCH, 512, NG, 2, 128).transpose(0, 4, 2, 3, 1))
        h8 = _q8(xr)
        l8 = _q8(xr - h8.astype(np.float32))
        xhs.append(h8.reshape(NCH, 128, NG * 1024))
        xls.append(l8.reshape(NCH, 128, NG * 1024))

    in_maps = []
    for c in range(NCORES):
        b, g = c // 2, c % 2
        sl = slice(g * FPC, (g + 1) * FPC)

        def pack_qk(w):
            # [d, col] -> [part, p, g, i, f]: d = 256g+128i+part, col = 128p+f
            # and f (psum partition) reordered to (j, h, s):
            # feature-within-pair = 64h + 32j + s  ->  f = 64j + 32h + s
            wr = (w * WSC).reshape(NG, 2, 128, NPAIR, 2, 2, 32).transpose(
                2, 3, 0, 1, 5, 4, 6)
            hi = _q8(wr)
            lo = _q8(wr - hi.astype(np.float32))
            return (hi.reshape(128, 4096), lo.reshape(128, 4096))

        def pack_v(w):
            # [d, col] -> [part, g, i, f]: d = 256g+128i+part, col = f(512)
            wr = (w * WSC).reshape(NG, 2, 128, FPC).transpose(2, 0, 1, 3)
            hi = _q8(wr)
            lo = _q8(wr - hi.astype(np.float32))
            return (hi.reshape(128, 4096), lo.reshape(128, 4096))

        qh, ql = pack_qk(Wqkv[:, 0 * D:1 * D][:, sl])
        kh, kl = pack_qk(Wqkv[:, 1 * D:2 * D][:, sl])
        vh, vl = pack_v(Wqkv[:, 2 * D:3 * D][:, sl])

        in_maps.append({
            "xh": xhs[b], "xl": xls[b],
            "Wqh": qh, "Wql": ql, "Wkh": kh, "Wkl": kl, "Wvh": vh, "Wvl": vl,
            "BQ": _pack_bias(bqkv[0 * D:1 * D][sl]),
            "BK": _pack_bias(bqkv[1 * D:2 * D][sl]),
            "BV": np.ascontiguousarray(bqkv[2 * D:3 * D][sl]),
            "Wo": _rne11(Wo[sl, :]),
        })
    return in_maps


def kernel(x, Wqkv, bqkv, Wo, bo, mask=None, **_unused):
    x = np.asarray(x, dtype=np.float32)
    Wqkv = np.asarray(Wqkv, dtype=np.float32)
    bqkv = np.asarray(bqkv, dtype=np.float32)
    Wo = np.asarray(Wo, dtype=np.float32)
    bo = np.asarray(bo, dtype=np.float32)
    in_maps = make_in_maps(x, Wqkv, bqkv, Wo)
    last_err = None
    for _attempt in range(3):
        try:
            r = _get_runner()
            args = r.put_inputs(in_maps)
            res = r.run_np(args)
            break
        except Exception as e:  # transient device wedge: retry fresh
            last_err = e
            _CACHE.clear()
            import time
            time.sleep(5)
    else:
        raise last_err
    out = np.empty((B, T, D), dtype=np.float32)
    for b in range(B):
        out[b] = res[2 * b]["y"] + res[2 * b + 1]["y"] + bo
    return out



# revision 109
# speedup vs baseline: 1.0674x; 1.0674x over previous
"""Causal self-attention Trainium2 kernel, v3.

Sharding: 8 cores = 4 batches x 2 head-groups (8 heads each).

Per-core dataflow:
  - QKV projections as fp8e4 DoubleRow matmuls (256-feature contraction
    per instruction) with a hi/lo split of both x and W (host-prepared):
    q = xh@Wh + xl@Wh + xh@Wl  -- 3 DoubleRow passes = 6N cycles vs
    fp32r's 8N, with ~0.1% error.
  - q,k stored fp8 (DoubleRow scores per k-tile into PSUM).
  - exp on ACT writes P^T directly as bf16; causal triangle zeroed on
    GPSIMD (affine_select).
  - PV flipped: out[q, f] = P^T[k, q-tile].T @ Vaug[k, 65] per q-tile,
    full 128-partition PSUM use (half the PE time of the [65, q]
    orientation); the ones-column of Vaug accumulates the softmax
    denominator at f=64.
  - normalize pre-transpose on DVE (reciprocal of col 64, broadcast
    multiply), then one XBAR dma_start_transpose per (chunk, pair)
    flips the normalized bf16 tiles into outT[128 feat, T].
  - y = outT.T @ Wo in bf16 (partial; host sums the 2 head-groups).

Scheduling: projection chains for chunk ch+1 and output-projection tiles
for chunk ch-1 are interleaved between attention k-tiles of chunk ch so
the PE never idles while ACT paces the softmax.
"""
import numpy as np

B, T, D, H = 4, 2048, 1024, 16
HD = D // H            # 64
NCORES = 8
HPC = 8                # heads per core
FPC = HPC * HD         # 512 feature cols per core
NPAIR = HPC // 2       # 4 head pairs
NG = 4                 # fp8 DoubleRow contraction groups (256 feats each)
KT = T // 128          # 16 k-tiles
NCH = T // 512         # 4 q-chunks
WSC = 50.0             # host weight scaling before fp8 quantization
QKS = 16.0             # q/k fp8 storage scale
VSTR = 65              # per-k-tile stride in vaug free dim
HSTR = KT * VSTR       # per-head stride in vaug free dim

_CACHE = {}


def _build():
    import concourse.mybir as mybir
    import concourse.tile as tile
    from concourse import bacc
    from contextlib import ExitStack

    f32 = mybir.dt.float32
    f32r = mybir.dt.float32r
    bf16 = mybir.dt.bfloat16
    f8 = mybir.dt.float8e4
    DR = mybir.MatmulPerfMode.DoubleRow
    Exp = mybir.ActivationFunctionType.Exp
    Alu = mybir.AluOpType

    nc = bacc.Bacc("TRN2", target_bir_lowering=False, debug=False,
                   num_devices=NCORES)
    # x hi/lo fp8, repacked host-side as [ch, 128, g, i, tok]:
    # feature = 256*g + 128*i + partition
    xh = nc.dram_tensor("xh", [NCH, 128, NG * 1024], f8, kind="ExternalInput")
    xl = nc.dram_tensor("xl", [NCH, 128, NG * 1024], f8, kind="ExternalInput")
    # fp8 weights packed in load-order: per pair p the (Wqh, Wql, Wkh,
    # Wkl) column-quarters, then Wvh, Wvl -- few big DMAs, JIT by pair
    Wall = nc.dram_tensor("Wall", [128, 6 * 4096], f8, kind="ExternalInput")
    BQ = nc.dram_tensor("BQ", [128, NPAIR], f32, kind="ExternalInput")
    BK = nc.dram_tensor("BK", [128, NPAIR], f32, kind="ExternalInput")
    BV = nc.dram_tensor("BV", [FPC], f32, kind="ExternalInput")
    Wo = nc.dram_tensor("Wo", [FPC, D], bf16, kind="ExternalInput")
    y = nc.dram_tensor("y", [T, D], f32, kind="ExternalOutput")

    with tile.TileContext(nc) as tc, ExitStack() as es:
        pers = es.enter_context(tc.tile_pool(name="pers", bufs=1))
        xsp = es.enter_context(tc.tile_pool(name="xsp", bufs=3))
        # pt tiles live from their exp until the PV drain carried into
        # the next column
        ptp = es.enter_context(tc.tile_pool(name="ptp", bufs=22))
        nrm = es.enter_context(tc.tile_pool(name="nrm", bufs=2))
        obp = es.enter_context(tc.tile_pool(name="obp", bufs=5))
        stgp = es.enter_context(tc.tile_pool(name="stgp", bufs=4))
        psA = es.enter_context(tc.tile_pool(name="psA", bufs=2, space="PSUM"))
        psS = es.enter_context(tc.tile_pool(name="psS", bufs=2, space="PSUM"))
        psO = es.enter_context(tc.tile_pool(name="psO", bufs=1, space="PSUM"))

        ws_sb = pers.tile([128, 6 * 4096], f8, tag="ws", name="ws")
        bq_sb = pers.tile([128, NPAIR], f32, tag="bq")
        bk_sb = pers.tile([128, NPAIR], f32, tag="bk")
        bv_row = pers.tile([1, FPC], f32, tag="bvr")
        bv_bc = pers.tile([128, FPC], f32, tag="bvb")
        wo_sb = pers.tile([128, NPAIR * D], bf16, tag="wo")
        vaug = pers.tile([128, HPC * HSTR], bf16, tag="vaug")
        # q/k in fp8 for DoubleRow scores: tile u holds pairs (2u, 2u+1);
        # partition = 64*(pr%2) + 32*head + hd%32, free = (hd-half j,
        # q-or-k, token) -- one tile so each remap DMA moves q and k
        qkT8 = [pers.tile([128, 2, 2, T], f8, tag=f"qkT8{u}",
                          name=f"qkT8{u}") for u in range(2)]
        # (h,f)-major attention outputs, chunk-major so each chunk's
        # transpose destination is one contiguous per-partition run
        outT = pers.tile([128, NCH, NPAIR, 512], bf16, tag="oT", name="oT")

        vaug4 = vaug[:].rearrange("p (h k x) -> p h k x", h=HPC, k=KT)

        xtiles = {}

        def emit_xdma(ch):
            xh_sb = xsp.tile([128, NG * 1024], f8, tag="xh", name="xh_sb")
            xl_sb = xsp.tile([128, NG * 1024], f8, tag="xl", name="xl_sb")
            if ch == 0:
                half = NG * 512
                nc.sync.dma_start(xh_sb[:, 0:half], xh[ch][:, 0:half])
                nc.sync.dma_start(xl_sb[:, 0:half], xl[ch][:, 0:half])
                nc.sync.dma_start(xh_sb[:, half:], xh[ch][:, half:])
                nc.sync.dma_start(xl_sb[:, half:], xl[ch][:, half:])
            else:
                nc.sync.dma_start(xh_sb[:], xh[ch])
                nc.sync.dma_start(xl_sb[:], xl[ch])
            xtiles[ch] = [xx[:, g * 1024:(g + 1) * 1024].rearrange(
                "p (i t) -> p i t", i=2)
                for xx in (xh_sb, xl_sb) for g in range(NG)]

        def w_ap(sec, p, g):
            # pair blocks pack (qh, kh, ql, kl) -- interleaved-pass order
            base = p * 4096 + (0, 2, 1, 3)[sec] * 1024 + g * 256
            return ws_sb[:, base:base + 256].rearrange(
                "p (i f) -> p i f", i=2)

        def wv_ap(sec, g):
            base = 16384 + (sec - 4) * 4096 + g * 1024
            return ws_sb[:, base:base + 1024].rearrange(
                "p (i f) -> p i f", i=2)

        # (x-part, w-section) term order: xl last so its DMA can trail
        # at startup. psum partitions are ordered (hd-half j, head, hd%32)
        # by the host weight packing; both evacs write scaled fp8 into one
        # staging tile whose two j-halves are then DMA'd into the
        # partition-sliced qkT8 layout (one DMA moves q and k together).
        def proj_qk(ch, p, interleave=False):
            xg = xtiles[ch]
            stg = stgp.tile([128, 2, 512], f8, tag="stg", name="stg")
            pss = []
            passes = [[], []]
            for qk, sec in enumerate((0, 2)):
                ps = psA.tile([128, 512], f32, tag="psA", name="psqk")
                pss.append(ps)
                n = 0
                for (xt, ws) in ((0, sec), (0, sec + 1), (1, sec)):
                    emit_pass = []
                    for g in range(NG):
                        emit_pass.append(lambda ps=ps, ws=ws, p=p, g=g,
                                         xt=xt, n=n: nc.tensor.matmul(
                            ps[:], w_ap(ws, p, g), xg[xt * NG + g],
                            start=(n == 0), stop=(n == 3 * NG - 1),
                            perf_mode=DR))
                        n += 1
                    passes[qk].append(emit_pass)
            if interleave:
                # DMA-arrival order: q and k chains advance together so
                # the k evac is not serialized behind the whole q chain
                order = [(0, 0), (1, 0), (0, 1), (1, 1), (0, 2), (1, 2)]
            else:
                order = [(0, 0), (0, 1), (0, 2), (1, 0), (1, 1), (1, 2)]
            for qk, pi in order:
                for f in passes[qk][pi]:
                    f()
                if pi == 2:
                    bsb = (bq_sb, bk_sb)[qk]
                    nc.vector.tensor_scalar(
                        stg[:, qk, :], pss[qk][:], QKS / WSC,
                        bsb[:, p:p + 1], Alu.mult, Alu.add)
            u, e = p // 2, p % 2
            for j in range(2):
                nc.sync.dma_start(
                    qkT8[u][64 * e:64 * e + 64, j, :,
                            512 * ch:512 * (ch + 1)],
                    stg[64 * j:64 * j + 64, :, :])

        def proj_v(ch, tj):
            xg = xtiles[ch]
            lo = (tj - 4 * ch) * 128
            ps = psA.tile([128, FPC], f32, tag="psA", name="psv")
            n = 0
            for (xt, ws) in ((0, 4), (0, 5), (1, 4)):
                for g in range(NG):
                    nc.tensor.matmul(
                        ps[:], xg[xt * NG + g][:, :, lo:lo + 128],
                        wv_ap(ws, g),
                        start=(n == 0), stop=(n == 3 * NG - 1),
                        perf_mode=DR)
                    n += 1
            nc.vector.scalar_tensor_tensor(
                vaug4[:, :, tj, 0:64],
                ps[:].rearrange("p (h x) -> p h x", h=HPC),
                1.0 / WSC,
                bv_bc[:].rearrange("p (h x) -> p h x", h=HPC),
                Alu.mult, Alu.add)

        def proj_chunk_fill(ch):
            out = []
            for p in range(NPAIR):
                out.append(lambda p=p: proj_qk(ch, p))
            for tj in range(4 * ch, 4 * (ch + 1)):
                out.append(lambda tj=tj: proj_v(ch, tj))
            return out

        Copy = mybir.ActivationFunctionType.Copy

        def outproj_tile(tj, act_evac=False):
            ob = obp.tile([128, D], f32, tag="ob", name="ob")
            for n in range(2):
                ps = psA.tile([128, 512], f32, tag="psA", name="pspr")
                for p in range(NPAIR):
                    nc.tensor.matmul(
                        ps[:],
                        outT[:, tj // 4, p, 128 * (tj % 4):
                             128 * (tj % 4 + 1)],
                        wo_sb[:, p * D + 512 * n:p * D + 512 * (n + 1)],
                        start=(p == 0), stop=(p == NPAIR - 1))
                if act_evac:
                    # ACT evac + per-half writeback shortens the final
                    # drain; halves alternate between the two DMA queues
                    # so the issue rate doubles
                    nc.scalar.activation(ob[:, 512 * n:512 * (n + 1)],
                                         ps[:], Copy)
                    eng = nc.sync if n == 0 else nc.scalar
                    eng.dma_start(
                        y[128 * tj:128 * (tj + 1),
                          512 * n:512 * (n + 1)],
                        ob[:, 512 * n:512 * (n + 1)])
                else:
                    nc.vector.tensor_copy(ob[:, 512 * n:512 * (n + 1)],
                                          ps[:])
            if not act_evac:
                nc.sync.dma_start(y[128 * tj:128 * (tj + 1), :], ob[:])

        def chunk_transpose(ch, nsbc, p0, pn):
            # one XBAR transpose flips pn pairs' normalized [q, (h,f)]
            # staging into the (h,f)-major outT chunk section
            nc.sync.dma_start_transpose(
                outT[:, ch, p0:p0 + pn, :].rearrange(
                    "p a (b c) -> p (a b) c", c=128),
                nsbc[:, p0:p0 + pn].rearrange("p a b h x -> p (a b h x)"))

        def attn_column(p, ch, fill, nsbc, tailjobs=None, carry_in=None,
                        carry_out=False, fill_first=False):
            """fill: iterator of emitters to interleave between k-tiles.

            carry_out: skip the drain+normalize and return them as thunks;
            carry_in: the previous column's thunks, replayed across this
            column's k-tiles (chunk-0 pipelining); fill_first: emit each
            k-tile's fill before its scores (chunk 0, where scores wait
            on the remap anyway)."""
            # flipped PV accumulators: [128 q, qi, 65] per head, padded to
            # 128 cols per qi-subtile so accumulation regions stay inside
            # one PSUM bank
            po = [psO.tile([128, 4, 128], f32, tag=f"po{h}", name=f"po{h}")
                  for h in range(2)]
            nk = 4 * ch + 4

            def emit_pv(qi, pend):
                # one PSUM bank holds all four qi accumulators, so each
                # (h, qi) group's matmuls must be consecutive: interleaved
                # open groups within a bank reset each other on hardware
                for h in range(2):
                    for kt in range(4 * ch + qi + 1):
                        nc.tensor.matmul(
                            po[h][:, qi, 0:65],
                            pend[kt][:, h, 128 * qi:128 * (qi + 1)],
                            vaug4[:, 2 * p + h, kt, :],
                            start=(kt == 0), stop=(kt == 4 * ch + qi),
                            skip_group_check=True)

            def emit_norm(qs, qn):
                # normalize po[:, qs:qs+qn, 0:64] by 1/po[..., 64] and
                # write bf16 (qi, h, f)-major staging for the transpose
                rec = nrm.tile([128, 2, 4], f32, tag="rec", name="rec")
                for h in range(2):
                    nc.vector.reciprocal(
                        rec[:, h, qs:qs + qn],
                        po[h][:, qs:qs + qn, 64:65].rearrange(
                            "p a b -> p (a b)"))
                    nc.vector.tensor_mul(
                        nsbc[:, p, qs:qs + qn, h, :],
                        po[h][:, qs:qs + qn, 0:64],
                        rec[:, h, qs:qs + qn].rearrange(
                            "p (a b) -> p a b", b=1).broadcast_to(
                            (128, qn, 64)))

            def qi_transpose(qi):
                nc.sync.dma_start_transpose(
                    outT[:, ch, p, 128 * qi:128 * (qi + 1)],
                    nsbc[:, p, qi].rearrange("p h x -> p (h x)"))

            pend = []
            for kt in range(nk):
                dg = kt - 4 * ch
                qo = max(dg, 0) * 128
                if fill_first and kt == 0:
                    f = next(fill, None)
                    if f:
                        f()
                pt = ptp.tile([128, 2, 512], bf16, tag="pt", name="pt")
                ss = psS.tile([128, 2, 512], f32, tag="psS", name="ss")
                u, e = p // 2, p % 2
                for h in range(2):
                    b0 = 64 * e + 32 * h
                    nc.tensor.matmul(
                        ss[:, h, qo:],
                        qkT8[u][b0:b0 + 32, :, 1,
                                128 * kt:128 * (kt + 1)],
                        qkT8[u][b0:b0 + 32, :, 0,
                                512 * ch + qo:512 * (ch + 1)],
                        start=True, stop=True, perf_mode=DR,
                        tile_position=(b0, 0))
                nc.scalar.activation(pt[:, :, qo:], ss[:, :, qo:],
                                     Exp, scale=0.125 / (QKS * QKS))
                if dg >= 0:
                    # zero the upper (q < k) triangle of the diagonal block
                    blk = pt[:, :, qo:qo + 128]
                    nc.gpsimd.affine_select(
                        out=blk, in_=blk,
                        compare_op=mybir.AluOpType.is_ge,
                        fill=0.0, base=0,
                        pattern=[[0, 2], [1, 128]],
                        channel_multiplier=-1)
                if not (fill_first and kt == 0):
                    f = next(fill, None)
                    if f:
                        f()
                if carry_in and kt >= 1:
                    carry_in.pop(0)()
                pend.append(pt)
                if tailjobs is not None and kt >= nk - 3:
                    # the qi-th accumulator only needs exps through k-tile
                    # 4ch+qi: drain it as soon as its last exp can land
                    qi = kt - (nk - 3)
                    emit_pv(qi, pend)
                    emit_norm(qi, 1)
                    qi_transpose(qi)
                    tailjobs[qi]()
            while carry_in:
                carry_in.pop(0)()
            if carry_out:
                # consume this column's drain-slot fills, then hand the PV
                # groups + normalize to the next column's k-tile loop
                for _ in range(4):
                    f = next(fill, None)
                    if f:
                        f()
                out = [lambda qi=qi: emit_pv(qi, pend) for qi in range(4)]
                out.append(lambda: emit_norm(0, 4))
                return out
            if tailjobs is None:
                for qi in range(4):
                    f = next(fill, None)
                    if f:
                        f()
                    emit_pv(qi, pend)
                emit_norm(0, 4)
            else:
                # last column: quarters 0..2 drained inside the k-tile
                # loop; only the last quarter remains
                emit_pv(3, pend)
                emit_norm(3, 1)
                qi_transpose(3)
                tailjobs[3]()

        # ---------------- schedule ----------------
        # startup DMA order: only what the first q/k chain and its scores
        # need -- everything else defers behind the first remap
        xh_sb = xsp.tile([128, NG * 1024], f8, tag="xh", name="xh_sb")
        xl_sb = xsp.tile([128, NG * 1024], f8, tag="xl", name="xl_sb")
        # minimal first-chain working set in pass order: the (qh, kh)
        # half-block feeds passes 1-2, (ql, kl) passes 3-4 -- few issues
        # (the 650ns/DMA queue rate is the startup floor)
        nc.sync.dma_start(ws_sb[:, 0:2048], Wall[:, 0:2048])
        nc.sync.dma_start(xh_sb[:], xh[0])
        nc.sync.dma_start(ws_sb[:, 2048:4096], Wall[:, 2048:4096])
        nc.sync.dma_start(xl_sb[:], xl[0])
        nc.sync.dma_start(bq_sb[:], BQ[:])
        nc.sync.dma_start(bk_sb[:], BK[:])
        xtiles[0] = [xx[:, g * 1024:(g + 1) * 1024].rearrange(
            "p (i t) -> p i t", i=2)
            for xx in (xh_sb, xl_sb) for g in range(NG)]
        nc.vector.memset(vaug4[:, :, :, 64:65], 1.0)

        # pair-1 weights ahead of the remaps (its chain follows at once)
        nc.sync.dma_start(ws_sb[:, 4096:8192], Wall[:, 4096:8192])

        # chunk-0 pair-0 projection only -- the first attention column
        # starts right behind it; the remaining loads are spread as fills
        # in need order so no critical remap queues behind bulk bytes
        proj_qk(0, 0, interleave=True)

        # pair-2..3 weights behind the first remaps, ahead of their chains
        # in column (0,0)'s later fills
        for c0 in range(8192, 16384, 2048):
            nc.sync.dma_start(ws_sb[:, c0:c0 + 2048], Wall[:, c0:c0 + 2048])

        xh1_sb = xsp.tile([128, NG * 1024], f8, tag="xh", name="xh_sb")
        xl1_sb = xsp.tile([128, NG * 1024], f8, tag="xl", name="xl_sb")
        xtiles[1] = [xx[:, g * 1024:(g + 1) * 1024].rearrange(
            "p (i t) -> p i t", i=2)
            for xx in (xh1_sb, xl1_sb) for g in range(NG)]

        def wall_piece(c0):
            nc.sync.dma_start(ws_sb[:, c0:c0 + 2048], Wall[:, c0:c0 + 2048])

        def bv_load():
            nc.sync.dma_start(bv_row[:], BV[:])
            nc.gpsimd.partition_broadcast(bv_bc[:], bv_row[:])

        bulk = (
            [lambda c0=c0: wall_piece(c0)
             for c0 in range(16384, 24576, 2048)] +      # Wv
            [bv_load] +
            [lambda c0=c0: nc.sync.dma_start(
                xh1_sb[:, c0:c0 + 2048], xh[1][:, c0:c0 + 2048])
             for c0 in (0, 2048)] +
            [lambda c0=c0: nc.sync.dma_start(
                xl1_sb[:, c0:c0 + 2048], xl[1][:, c0:c0 + 2048])
             for c0 in (0, 2048)] +
            [lambda t=t: nc.sync.dma_start(
                wo_sb[:, t * D:(t + 1) * D],
                Wo[128 * t:128 * (t + 1), :])
             for t in range(NPAIR)])

        # ---- deadline-packed fill assignment over the k-tile slots ----
        # each non-tail column also exposes its 4 PV-drain positions as
        # fill slots (the drain interleaves next(fill) before each PV)
        last = NCH - 1
        slots = []           # (ch, p, s) in emission order
        slot_of = {}
        for ch in range(NCH):
            for p in range(NPAIR):
                ns = 4 * ch + 4
                if not (ch == last and p == NPAIR - 1):
                    ns += 4
                for s in range(ns):
                    slot_of[(ch, p, s)] = len(slots)
                    slots.append((ch, p, s))
        nslots = len(slots)
        chunk_start = {ch: slot_of[(ch, 0, 0)] for ch in range(NCH)}
        chunk_end = {ch: slot_of[(ch, NPAIR - 1, 4 * ch + 3)]
                     for ch in range(NCH)}

        nsbcs = {}
        items = []  # (deadline, avail, fn, label)
        # chunk 0: V weights + deferred setup DMAs first (they jump no
        # queue), then the remaining q/k chains, V chains at the drains
        # chunk 0: pinned layout -- loads stream in need order, chains one
        # column ahead, V chains just before their carried-PV replays
        pinned = {
            (0, 0, 0): ("qk(0,1)", lambda: proj_qk(0, 1)),
            (0, 0, 1): ("qk(0,2)", lambda: proj_qk(0, 2)),
            (0, 0, 2): ("qk(0,3)", lambda: proj_qk(0, 3)),
            (0, 0, 3): ("wva", bulk[0]),
            (0, 0, 4): ("wvb", bulk[1]),
            (0, 0, 5): ("wvc", bulk[2]),
            (0, 0, 6): ("wvd", bulk[3]),
            (0, 0, 7): ("bv", bulk[4]),
            (0, 1, 0): ("v(0,0)", lambda: proj_v(0, 0)),
            (0, 1, 1): ("x1ha", bulk[5]),
            (0, 1, 2): ("v(0,1)", lambda: proj_v(0, 1)),
            (0, 1, 3): ("x1hb", bulk[6]),
            (0, 1, 4): ("v(0,2)", lambda: proj_v(0, 2)),
            (0, 1, 5): ("x1la", bulk[7]),
            (0, 1, 6): ("v(0,3)", lambda: proj_v(0, 3)),
            (0, 1, 7): ("x1lb", bulk[8]),
            (0, 2, 0): ("woa", bulk[9]),
            (0, 2, 1): ("wob", bulk[10]),
            (0, 2, 2): ("woc", bulk[11]),
            (0, 2, 3): ("wod", bulk[12]),
        }
        for key, (lb, fn) in pinned.items():
            s = slot_of[key]
            items.append((s, s, fn, lb))
        # chunks 1+: q/k chains land one column ahead of use; V chains and
        # the p>=1 q/k chains ride the target chunk's own ACT slack
        for ch in range(1, NCH):
            av = chunk_start[ch - 1]
            # chain->evac->remap->sem is ~5.5us: give 6 slots of margin
            # (more in ch1 whose slots are shorter)
            mg = 10 if ch == 1 else 6
            items.append((slot_of[(ch, 0, 0)] - mg, av,
                          lambda ch=ch: proj_qk(ch, 0), f"qk({ch},0)"))
            # chunk-3 projections pull forward into chunk 2's PE slack so
            # their DVE evacs never queue ahead of the normalizes that the
            # PSUM-pool rotation waits on
            for p in range(1, NPAIR):
                if ch == 3:
                    dl = slot_of[(2, 3, 4 * p)] if p < 3 \
                        else slot_of[(3, 0, 2)]
                    av2 = chunk_start[2]
                else:
                    dl = slot_of[(ch, p, 0)] - (10 if ch == 1 else 6)
                    av2 = chunk_start[ch] if ch == 2 else av
                items.append((dl, av2, lambda ch=ch, p=p: proj_qk(ch, p),
                              f"qk({ch},{p})"))
            for tj in range(4 * ch, 4 * (ch + 1)):
                # the PV group needing this k-tile replays in column
                # (ch, 1) at k-tile (tj - 4ch) + 1
                if ch == 3:
                    dl = slot_of[(3, 0, 8 + 2 * (tj - 12))]
                    av2 = chunk_start[2]
                else:
                    dl = slot_of[(ch, 1, tj - 4 * ch + 1)] - 1
                    av2 = chunk_start[ch] if ch == 2 else av
                items.append((dl, av2, lambda ch=ch, tj=tj: proj_v(ch, tj),
                              f"v({ch},{tj})"))
        for ch in range(NCH - 1):
            # chunk transposes fire after the carried normalize of the
            # chunk's last pair, which replays at k-tile 5 of the next
            # chunk's first column
            av = slot_of[(ch + 1, 0, 6)]
            items.append((av + 1, av,
                          lambda ch=ch: chunk_transpose(
                              ch, nsbcs[ch], 0, NPAIR), f"T({ch})"))
        # output tiles spread over late-chunk columns (clustering them at
        # the end starves the PE mid-chunk and jams the DMA queue); the
        # chunk-2 tiles ride the ACT-paced slack of the last two columns
        for tj in range(4):
            av = slot_of[(1, 1, 0)]
            items.append((slot_of[(2, tj, 10)], av,
                          lambda tj=tj: outproj_tile(tj), f"op({tj})"))
        for tj in range(4, 8):
            av = slot_of[(2, 1, 0)]
            items.append((slot_of[(3, tj - 4, 10)], av,
                          lambda tj=tj: outproj_tile(tj), f"op({tj})"))
        for tj, key in ((8, (3, 2, 10)), (9, (3, 2, 13)),
                        (10, (3, 3, 7)), (11, (3, 3, 10))):
            items.append((slot_of[key], slot_of[key],
                          lambda tj=tj: outproj_tile(tj), f"op({tj})"))
        # the last chunk's pairs-0..2 transpose fires right after the
        # carried normalize of (3,2) lands, clearing the tail's DMA path
        items.append((slot_of[(3, 3, 6)], slot_of[(3, 3, 6)],
                      lambda: chunk_transpose(3, nsbcs[3], 0, NPAIR - 1),
                      "T3p012"))

        assigned = {}
        labels = {}
        for dl, av, fn, lb in sorted(items, key=lambda it: -it[0]):
            s = min(dl, nslots - 1)
            while s >= av and s in assigned:
                s -= 1
            if s < av:
                s = av
                while s in assigned:
                    s += 1
            assigned[s] = fn
            labels[s] = lb
        import os
        if os.environ.get("KERNEL_DEBUG_SLOTS"):
            for s in sorted(labels):
                print(f"slot {s:3d} {slots[s]} <- {labels[s]}")

        fills = {}
        for s, fn in assigned.items():
            fills.setdefault(s, []).append(fn)

        def _slotfill(seq):
            for fl in seq:
                yield (lambda fl=fl: [f() for f in fl]) if fl else None
            while True:
                yield None

        carry = None
        for ch in range(NCH):
            nsbc = nrm.tile([128, NPAIR, 4, 2, 64], bf16, tag="nsbc",
                            name="nsbc")
            nsbcs[ch] = nsbc
            if 0 < ch and ch + 1 < NCH:
                emit_xdma(ch + 1)
            for p in range(NPAIR):
                ns = 4 * ch + 4
                if not (ch == last and p == NPAIR - 1):
                    ns += 4
                seq = []
                for s in range(ns):
                    fl = fills.get(slot_of[(ch, p, s)], [])
                    seq.append(fl)
                if ch == 0:
                    # carry columns pull two fill slots per k-tile
                    seq = [seq[2 * j] + seq[2 * j + 1] for j in range(4)]
                it = _slotfill(seq)
                tailjobs = None
                if ch == last and p == NPAIR - 1:
                    tailjobs = [
                        lambda tj=tj: outproj_tile(tj, act_evac=True)
                        for tj in range(4 * last, 4 * last + 4)]
                if ch == last and p == NPAIR - 1:
                    attn_column(p, ch, it, nsbc, tailjobs, carry_in=carry)
                    carry = None
                else:
                    carry = attn_column(p, ch, it, nsbc, carry_in=carry,
                                        carry_out=True,
                                        fill_first=(ch == 0))
    nc.compile()
    return nc


class _Runner:
    def __init__(self, nc):
        import jax
        from jax.sharding import Mesh, PartitionSpec, NamedSharding
        from jax.experimental.shard_map import shard_map
        import concourse.mybir as mybir
        from concourse.bass2jax import (_bass_exec_p, partition_id_tensor,
                                        install_neuronx_cc_hook)
        install_neuronx_cc_hook()
        self.jax = jax
        part = nc.partition_id_tensor.name if nc.partition_id_tensor else None
        in_names, out_names, out_avals = [], [], []
        for alloc in nc.m.functions[0].allocations:
            if not isinstance(alloc, mybir.MemoryLocationSet):
                continue
            name = alloc.memorylocations[0].name
            if alloc.kind == "ExternalInput":
                if name != part:
                    in_names.append(name)
            elif alloc.kind == "ExternalOutput":
                out_names.append(name)
                out_avals.append(jax.core.ShapedArray(
                    tuple(alloc.tensor_shape), mybir.dt.np(alloc.dtype)))
        self.in_names, self.out_names, self.out_avals = in_names, out_names, out_avals
        all_in = list(in_names) + list(out_names) + ([part] if part else [])

        def _body(*args):
            ops = list(args)
            if part:
                ops.append(partition_id_tensor())
            return tuple(_bass_exec_p.bind(
                *ops, out_avals=tuple(out_avals), in_names=tuple(all_in),
                out_names=tuple(out_names), lowering_input_output_aliases=(),
                sim_require_finite=True, sim_require_nnan=True, nc=nc))

        devices = jax.devices()[:NCORES]
        mesh = Mesh(np.asarray(devices), ("core",))
        nin = len(in_names) + len(out_names)
        self.fn = jax.jit(
            shard_map(_body, mesh=mesh,
                      in_specs=(PartitionSpec("core"),) * nin,
                      out_specs=(PartitionSpec("core"),) * len(out_names),
                      check_rep=False),
            keep_unused=True)
        self.sharding = NamedSharding(mesh, PartitionSpec("core"))

    def put_inputs(self, in_maps):
        args = []
        for name in self.in_names:
            cat = np.concatenate([np.asarray(m[name]) for m in in_maps], axis=0)
            args.append(self.jax.device_put(cat, self.sharding))
        for av in self.out_avals:
            z = np.zeros((NCORES * av.shape[0], *av.shape[1:]), av.dtype)
            args.append(self.jax.device_put(z, self.sharding))
        return args

    def run_np(self, args):
        outs = self.fn(*args)
        return [
            {n: np.asarray(outs[i]).reshape(NCORES, *self.out_avals[i].shape)[c]
             for i, n in enumerate(self.out_names)}
            for c in range(NCORES)
        ]


def _get_runner():
    if "r" not in _CACHE:
        nc = _build()
        _CACHE["nc"] = nc
        _CACHE["r"] = _Runner(nc)
    return _CACHE["r"]


def _rne11(a):
    """Round fp32 to 11 mantissa bits, round-to-nearest-even (= hw fp32r)."""
    ai = np.ascontiguousarray(a, dtype=np.float32).view(np.uint32).astype(np.uint64)
    lsb = (ai >> 12) & 1
    out = (((ai + 2047 + lsb) >> 12) << 12).astype(np.uint32)
    return out.view(np.float32)


def _q8(a):
    import ml_dtypes
    a = np.clip(np.asarray(a, np.float32), -240.0, 240.0)
    return a.astype(ml_dtypes.float8_e4m3)


def _pack_bias(b):
    # [512] -> [128, NPAIR] with the (j, h, s) psum-partition order,
    # pre-scaled by the q/k fp8 storage scale
    br = (b * QKS).reshape(NPAIR, 2, 2, 32).transpose(2, 1, 3, 0)
    return np.ascontiguousarray(br.reshape(128, NPAIR).astype(np.float32))


def make_in_maps(x, Wqkv, bqkv, Wo, bo=None, mask=None):
    # x repack: [ch, part, g, i, tok], feature = 256g + 128i + part
    xhs, xls = [], []
    for b in range(B):
        xr = np.ascontiguousarray(
            x[b].reshape(NCH, 512, NG, 2, 128).transpose(0, 4, 2, 3, 1))
        h8 = _q8(xr)
        l8 = _q8(xr - h8.astype(np.float32))
        xhs.append(h8.reshape(NCH, 128, NG * 1024))
        xls.append(l8.reshape(NCH, 128, NG * 1024))

    in_maps = []
    for c in range(NCORES):
        b, g = c // 2, c % 2
        sl = slice(g * FPC, (g + 1) * FPC)

        def pack_qk(w):
            # [d, col] -> [part, p, g, i, f]: d = 256g+128i+part, col = 128p+f
            # and f (psum partition) reordered to (j, h, s):
            # feature-within-pair = 64h + 32j + s  ->  f = 64j + 32h + s
            wr = (w * WSC).reshape(NG, 2, 128, NPAIR, 2, 2, 32).transpose(
                2, 3, 0, 1, 5, 4, 6)
            hi = _q8(wr)
            lo = _q8(wr - hi.astype(np.float32))
            return (hi.reshape(128, 4096), lo.reshape(128, 4096))

        def pack_v(w):
            # [d, col] -> [part, g, i, f]: d = 256g+128i+part, col = f(512)
            wr = (w * WSC).reshape(NG, 2, 128, FPC).transpose(2, 0, 1, 3)
            hi = _q8(wr)
            lo = _q8(wr - hi.astype(np.float32))
            return (hi.reshape(128, 4096), lo.reshape(128, 4096))

        qh, ql = pack_qk(Wqkv[:, 0 * D:1 * D][:, sl])
        kh, kl = pack_qk(Wqkv[:, 1 * D:2 * D][:, sl])
        vh, vl = pack_v(Wqkv[:, 2 * D:3 * D][:, sl])
        # pack in load order: per pair the (qh, ql, kh, kl) column
        # quarters, then vh, vl
        wall = np.concatenate(
            [w[:, p * 1024:(p + 1) * 1024]
             for p in range(NPAIR) for w in (qh, kh, ql, kl)] + [vh, vl],
            axis=1)

        import ml_dtypes
        in_maps.append({
            "xh": xhs[b], "xl": xls[b],
            "Wall": np.ascontiguousarray(wall),
            "BQ": _pack_bias(bqkv[0 * D:1 * D][sl]),
            "BK": _pack_bias(bqkv[1 * D:2 * D][sl]),
            "BV": np.ascontiguousarray(bqkv[2 * D:3 * D][sl]),
            "Wo": np.ascontiguousarray(Wo[sl, :]).astype(ml_dtypes.bfloat16),
        })
    return in_maps


def kernel(x, Wqkv, bqkv, Wo, bo, mask=None, **_unused):
    x = np.asarray(x, dtype=np.float32)
    Wqkv = np.asarray(Wqkv, dtype=np.float32)
    bqkv = np.asarray(bqkv, dtype=np.float32)
    Wo = np.asarray(Wo, dtype=np.float32)
    bo = np.asarray(bo, dtype=np.float32)
    in_maps = make_in_maps(x, Wqkv, bqkv, Wo)
    last_err = None
    for _attempt in range(3):
        try:
            r = _get_runner()
            args = r.put_inputs(in_maps)
            res = r.run_np(args)
            break
        except Exception as e:  # transient device wedge: retry fresh
            last_err = e
            _CACHE.clear()
            import time
            time.sleep(5)
    else:
        raise last_err
    out = np.empty((B, T, D), dtype=np.float32)
    for b in range(B):
        out[b] = res[2 * b]["y"] + res[2 * b + 1]["y"] + bo
    return out



# revision 123
# speedup vs baseline: 1.0744x; 1.0066x over previous
"""Causal self-attention Trainium2 kernel, v3.

Sharding: 8 cores = 4 batches x 2 head-groups (8 heads each).

Per-core dataflow:
  - QKV projections as fp8e4 DoubleRow matmuls (256-feature contraction
    per instruction) with a hi/lo split of both x and W (host-prepared):
    q = xh@Wh + xl@Wh + xh@Wl  -- 3 DoubleRow passes = 6N cycles vs
    fp32r's 8N, with ~0.1% error.
  - q,k stored fp8 (DoubleRow scores per k-tile into PSUM).
  - exp on ACT writes P^T directly as bf16; causal triangle zeroed on
    GPSIMD (affine_select).
  - PV flipped: out[q, f] = P^T[k, q-tile].T @ Vaug[k, 65] per q-tile,
    full 128-partition PSUM use (half the PE time of the [65, q]
    orientation); the ones-column of Vaug accumulates the softmax
    denominator at f=64.
  - normalize pre-transpose on DVE (reciprocal of col 64, broadcast
    multiply), then one XBAR dma_start_transpose per (chunk, pair)
    flips the normalized bf16 tiles into outT[128 feat, T].
  - y = outT.T @ Wo in bf16 (partial; host sums the 2 head-groups).

Scheduling: projection chains for chunk ch+1 and output-projection tiles
for chunk ch-1 are interleaved between attention k-tiles of chunk ch so
the PE never idles while ACT paces the softmax.
"""
import numpy as np

B, T, D, H = 4, 2048, 1024, 16
HD = D // H            # 64
NCORES = 8
HPC = 8                # heads per core
FPC = HPC * HD         # 512 feature cols per core
NPAIR = HPC // 2       # 4 head pairs
NG = 4                 # fp8 DoubleRow contraction groups (256 feats each)
KT = T // 128          # 16 k-tiles
NCH = T // 512         # 4 q-chunks
WSC = 50.0             # host weight scaling before fp8 quantization
QKS = 16.0             # q/k fp8 storage scale
VSTR = 65              # per-k-tile stride in vaug free dim
HSTR = KT * VSTR       # per-head stride in vaug free dim

_CACHE = {}


def _build():
    import concourse.mybir as mybir
    import concourse.tile as tile
    from concourse import bacc
    from contextlib import ExitStack

    f32 = mybir.dt.float32
    f32r = mybir.dt.float32r
    bf16 = mybir.dt.bfloat16
    f8 = mybir.dt.float8e4
    DR = mybir.MatmulPerfMode.DoubleRow
    Exp = mybir.ActivationFunctionType.Exp
    Alu = mybir.AluOpType

    nc = bacc.Bacc("TRN2", target_bir_lowering=False, debug=False,
                   num_devices=NCORES)
    # x hi/lo fp8, repacked host-side as [ch, 128, g, i, tok]:
    # feature = 256*g + 128*i + partition
    xh = nc.dram_tensor("xh", [NCH, 128, NG * 1024], f8, kind="ExternalInput")
    xl = nc.dram_tensor("xl", [NCH, 128, NG * 1024], f8, kind="ExternalInput")
    # fp8 weights packed in load-order: per pair p the (Wqh, Wql, Wkh,
    # Wkl) column-quarters, then Wvh, Wvl -- few big DMAs, JIT by pair
    Wall = nc.dram_tensor("Wall", [128, 6 * 4096], f8, kind="ExternalInput")
    BQ = nc.dram_tensor("BQ", [128, NPAIR], f32, kind="ExternalInput")
    BK = nc.dram_tensor("BK", [128, NPAIR], f32, kind="ExternalInput")
    BV = nc.dram_tensor("BV", [FPC], f32, kind="ExternalInput")
    Wo = nc.dram_tensor("Wo", [FPC, D], bf16, kind="ExternalInput")
    y = nc.dram_tensor("y", [T, D], f32, kind="ExternalOutput")

    with tile.TileContext(nc) as tc, ExitStack() as es:
        pers = es.enter_context(tc.tile_pool(name="pers", bufs=1))
        xsp = es.enter_context(tc.tile_pool(name="xsp", bufs=3))
        # pt tiles live from their exp until the PV drain carried into
        # the next column
        ptp = es.enter_context(tc.tile_pool(name="ptp", bufs=22))
        nrm = es.enter_context(tc.tile_pool(name="nrm", bufs=2))
        obp = es.enter_context(tc.tile_pool(name="obp", bufs=5))
        stgp = es.enter_context(tc.tile_pool(name="stgp", bufs=4))
        psA = es.enter_context(tc.tile_pool(name="psA", bufs=2, space="PSUM"))
        psS = es.enter_context(tc.tile_pool(name="psS", bufs=2, space="PSUM"))
        psO = es.enter_context(tc.tile_pool(name="psO", bufs=1, space="PSUM"))

        ws_sb = pers.tile([128, 6 * 4096], f8, tag="ws", name="ws")
        bq_sb = pers.tile([128, NPAIR], f32, tag="bq")
        bk_sb = pers.tile([128, NPAIR], f32, tag="bk")
        bv_row = pers.tile([1, FPC], f32, tag="bvr")
        bv_bc = pers.tile([128, FPC], f32, tag="bvb")
        wo_sb = pers.tile([128, NPAIR * D], bf16, tag="wo")
        vaug = pers.tile([128, HPC * HSTR], bf16, tag="vaug")
        # q/k in fp8 for DoubleRow scores: tile u holds pairs (2u, 2u+1);
        # partition = 64*(pr%2) + 32*head + hd%32, free = (hd-half j,
        # q-or-k, token) -- one tile so each remap DMA moves q and k
        qkT8 = [pers.tile([128, 2, 2, T], f8, tag=f"qkT8{u}",
                          name=f"qkT8{u}") for u in range(2)]
        # (h,f)-major attention outputs, chunk-major so each chunk's
        # transpose destination is one contiguous per-partition run
        outT = pers.tile([128, NCH, NPAIR, 512], bf16, tag="oT", name="oT")

        vaug4 = vaug[:].rearrange("p (h k x) -> p h k x", h=HPC, k=KT)

        xtiles = {}

        def emit_xdma(ch):
            xh_sb = xsp.tile([128, NG * 1024], f8, tag="xh", name="xh_sb")
            xl_sb = xsp.tile([128, NG * 1024], f8, tag="xl", name="xl_sb")
            if ch == 0:
                half = NG * 512
                nc.sync.dma_start(xh_sb[:, 0:half], xh[ch][:, 0:half])
                nc.sync.dma_start(xl_sb[:, 0:half], xl[ch][:, 0:half])
                nc.sync.dma_start(xh_sb[:, half:], xh[ch][:, half:])
                nc.sync.dma_start(xl_sb[:, half:], xl[ch][:, half:])
            else:
                nc.sync.dma_start(xh_sb[:], xh[ch])
                nc.sync.dma_start(xl_sb[:], xl[ch])
            xtiles[ch] = [xx[:, g * 1024:(g + 1) * 1024].rearrange(
                "p (i t) -> p i t", i=2)
                for xx in (xh_sb, xl_sb) for g in range(NG)]

        def w_ap(sec, p, g):
            # pair blocks pack (qh, kh, ql, kl) -- interleaved-pass order
            base = p * 4096 + (0, 2, 1, 3)[sec] * 1024 + g * 256
            return ws_sb[:, base:base + 256].rearrange(
                "p (i f) -> p i f", i=2)

        def wv_ap(sec, g):
            base = 16384 + (sec - 4) * 4096 + g * 1024
            return ws_sb[:, base:base + 1024].rearrange(
                "p (i f) -> p i f", i=2)

        # (x-part, w-section) term order: xl last so its DMA can trail
        # at startup. psum partitions are ordered (hd-half j, head, hd%32)
        # by the host weight packing; both evacs write scaled fp8 into one
        # staging tile whose two j-halves are then DMA'd into the
        # partition-sliced qkT8 layout (one DMA moves q and k together).
        def proj_qk(ch, p, interleave=False):
            xg = xtiles[ch]
            stg = stgp.tile([128, 2, 512], f8, tag="stg", name="stg")
            pss = []
            passes = [[], []]
            for qk, sec in enumerate((0, 2)):
                ps = psA.tile([128, 512], f32, tag="psA", name="psqk")
                pss.append(ps)
                n = 0
                for (xt, ws) in ((0, sec), (0, sec + 1), (1, sec)):
                    emit_pass = []
                    for g in range(NG):
                        emit_pass.append(lambda ps=ps, ws=ws, p=p, g=g,
                                         xt=xt, n=n: nc.tensor.matmul(
                            ps[:], w_ap(ws, p, g), xg[xt * NG + g],
                            start=(n == 0), stop=(n == 3 * NG - 1),
                            perf_mode=DR))
                        n += 1
                    passes[qk].append(emit_pass)
            if interleave:
                # DMA-arrival order: q and k chains advance together so
                # the k evac is not serialized behind the whole q chain
                order = [(0, 0), (1, 0), (0, 1), (1, 1), (0, 2), (1, 2)]
            else:
                order = [(0, 0), (0, 1), (0, 2), (1, 0), (1, 1), (1, 2)]
            for qk, pi in order:
                for f in passes[qk][pi]:
                    f()
                if pi == 2:
                    bsb = (bq_sb, bk_sb)[qk]
                    nc.vector.tensor_scalar(
                        stg[:, qk, :], pss[qk][:], QKS / WSC,
                        bsb[:, p:p + 1], Alu.mult, Alu.add)
            u, e = p // 2, p % 2
            for j in range(2):
                nc.sync.dma_start(
                    qkT8[u][64 * e:64 * e + 64, j, :,
                            512 * ch:512 * (ch + 1)],
                    stg[64 * j:64 * j + 64, :, :])

        def proj_v(ch, tj):
            xg = xtiles[ch]
            lo = (tj - 4 * ch) * 128
            ps = psA.tile([128, FPC], f32, tag="psA", name="psv")
            n = 0
            for (xt, ws) in ((0, 4), (0, 5), (1, 4)):
                for g in range(NG):
                    nc.tensor.matmul(
                        ps[:], xg[xt * NG + g][:, :, lo:lo + 128],
                        wv_ap(ws, g),
                        start=(n == 0), stop=(n == 3 * NG - 1),
                        perf_mode=DR)
                    n += 1
            nc.vector.scalar_tensor_tensor(
                vaug4[:, :, tj, 0:64],
                ps[:].rearrange("p (h x) -> p h x", h=HPC),
                1.0 / WSC,
                bv_bc[:].rearrange("p (h x) -> p h x", h=HPC),
                Alu.mult, Alu.add)

        def proj_chunk_fill(ch):
            out = []
            for p in range(NPAIR):
                out.append(lambda p=p: proj_qk(ch, p))
            for tj in range(4 * ch, 4 * (ch + 1)):
                out.append(lambda tj=tj: proj_v(ch, tj))
            return out

        Copy = mybir.ActivationFunctionType.Copy

        def outproj_tile(tj, act_evac=False):
            ob = obp.tile([128, D], f32, tag="ob", name="ob")
            for n in range(2):
                ps = psA.tile([128, 512], f32, tag="psA", name="pspr")
                for p in range(NPAIR):
                    nc.tensor.matmul(
                        ps[:],
                        outT[:, tj // 4, p, 128 * (tj % 4):
                             128 * (tj % 4 + 1)],
                        wo_sb[:, p * D + 512 * n:p * D + 512 * (n + 1)],
                        start=(p == 0), stop=(p == NPAIR - 1))
                if act_evac:
                    # ACT evac + per-half writeback shortens the final
                    # drain; halves alternate between the two DMA queues
                    # so the issue rate doubles
                    nc.scalar.activation(ob[:, 512 * n:512 * (n + 1)],
                                         ps[:], Copy)
                    eng = nc.sync if n == 0 else nc.scalar
                    eng.dma_start(
                        y[128 * tj:128 * (tj + 1),
                          512 * n:512 * (n + 1)],
                        ob[:, 512 * n:512 * (n + 1)])
                else:
                    nc.vector.tensor_copy(ob[:, 512 * n:512 * (n + 1)],
                                          ps[:])
            if not act_evac:
                nc.sync.dma_start(y[128 * tj:128 * (tj + 1), :], ob[:])

        def chunk_transpose(ch, nsbc, p0, pn):
            # one XBAR transpose flips pn pairs' normalized [q, (h,f)]
            # staging into the (h,f)-major outT chunk section
            nc.sync.dma_start_transpose(
                outT[:, ch, p0:p0 + pn, :].rearrange(
                    "p a (b c) -> p (a b) c", c=128),
                nsbc[:, p0:p0 + pn].rearrange("p a b h x -> p (a b h x)"))

        def attn_column(p, ch, fill, nsbc, tailjobs=None, carry_in=None,
                        carry_out=False, fill_first=False):
            """fill: iterator of emitters to interleave between k-tiles.

            carry_out: skip the drain+normalize and return them as thunks;
            carry_in: the previous column's thunks, replayed across this
            column's k-tiles (chunk-0 pipelining); fill_first: emit each
            k-tile's fill before its scores (chunk 0, where scores wait
            on the remap anyway)."""
            # flipped PV accumulators: [128 q, qi, 65] per head, padded to
            # 128 cols per qi-subtile so accumulation regions stay inside
            # one PSUM bank
            po = [psO.tile([128, 4, 128], f32, tag=f"po{h}", name=f"po{h}")
                  for h in range(2)]
            nk = 4 * ch + 4

            def emit_pv(qi, pend):
                # one PSUM bank holds all four qi accumulators, so each
                # (h, qi) group's matmuls must be consecutive: interleaved
                # open groups within a bank reset each other on hardware
                for h in range(2):
                    for kt in range(4 * ch + qi + 1):
                        nc.tensor.matmul(
                            po[h][:, qi, 0:65],
                            pend[kt][:, h, 128 * qi:128 * (qi + 1)],
                            vaug4[:, 2 * p + h, kt, :],
                            start=(kt == 0), stop=(kt == 4 * ch + qi),
                            skip_group_check=True)

            def emit_norm(qs, qn):
                # normalize po[:, qs:qs+qn, 0:64] by 1/po[..., 64] and
                # write bf16 (qi, h, f)-major staging for the transpose
                rec = nrm.tile([128, 2, 4], f32, tag="rec", name="rec")
                for h in range(2):
                    nc.vector.reciprocal(
                        rec[:, h, qs:qs + qn],
                        po[h][:, qs:qs + qn, 64:65].rearrange(
                            "p a b -> p (a b)"))
                    nc.vector.tensor_mul(
                        nsbc[:, p, qs:qs + qn, h, :],
                        po[h][:, qs:qs + qn, 0:64],
                        rec[:, h, qs:qs + qn].rearrange(
                            "p (a b) -> p a b", b=1).broadcast_to(
                            (128, qn, 64)))

            def qi_transpose(qi):
                nc.sync.dma_start_transpose(
                    outT[:, ch, p, 128 * qi:128 * (qi + 1)],
                    nsbc[:, p, qi].rearrange("p h x -> p (h x)"))

            pend = []
            for kt in range(nk):
                dg = kt - 4 * ch
                qo = max(dg, 0) * 128
                if fill_first and kt == 0:
                    f = next(fill, None)
                    if f:
                        f()
                pt = ptp.tile([128, 2, 512], bf16, tag="pt", name="pt")
                ss = psS.tile([128, 2, 512], f32, tag="psS", name="ss")
                u, e = p // 2, p % 2
                for h in range(2):
                    b0 = 64 * e + 32 * h
                    nc.tensor.matmul(
                        ss[:, h, qo:],
                        qkT8[u][b0:b0 + 32, :, 1,
                                128 * kt:128 * (kt + 1)],
                        qkT8[u][b0:b0 + 32, :, 0,
                                512 * ch + qo:512 * (ch + 1)],
                        start=True, stop=True, perf_mode=DR,
                        tile_position=(b0, 0))
                nc.scalar.activation(pt[:, :, qo:], ss[:, :, qo:],
                                     Exp, scale=0.125 / (QKS * QKS))
                if dg >= 0:
                    # zero the upper (q < k) triangle of the diagonal block
                    blk = pt[:, :, qo:qo + 128]
                    nc.gpsimd.affine_select(
                        out=blk, in_=blk,
                        compare_op=mybir.AluOpType.is_ge,
                        fill=0.0, base=0,
                        pattern=[[0, 2], [1, 128]],
                        channel_multiplier=-1)
                if not (fill_first and kt == 0):
                    f = next(fill, None)
                    if f:
                        f()
                if carry_in and kt >= 1:
                    carry_in.pop(0)()
                pend.append(pt)
                if tailjobs is not None and kt >= nk - 3:
                    # the qi-th accumulator only needs exps through k-tile
                    # 4ch+qi: drain it as soon as its last exp can land
                    qi = kt - (nk - 3)
                    emit_pv(qi, pend)
                    emit_norm(qi, 1)
                    qi_transpose(qi)
                    tailjobs[qi]()
            while carry_in:
                carry_in.pop(0)()
            if carry_out:
                # consume this column's drain-slot fills, then hand the PV
                # groups + normalize to the next column's k-tile loop
                for _ in range(4):
                    f = next(fill, None)
                    if f:
                        f()
                out = [lambda qi=qi: emit_pv(qi, pend) for qi in range(4)]
                out.append(lambda: emit_norm(0, 4))
                return out
            if tailjobs is None:
                for qi in range(4):
                    f = next(fill, None)
                    if f:
                        f()
                    emit_pv(qi, pend)
                emit_norm(0, 4)
            else:
                # last column: quarters 0..2 drained inside the k-tile
                # loop; only the last quarter remains
                emit_pv(3, pend)
                emit_norm(3, 1)
                qi_transpose(3)
                tailjobs[3]()

        # ---------------- schedule ----------------
        # startup DMA order: only what the first q/k chain and its scores
        # need -- everything else defers behind the first remap
        xh_sb = xsp.tile([128, NG * 1024], f8, tag="xh", name="xh_sb")
        xl_sb = xsp.tile([128, NG * 1024], f8, tag="xl", name="xl_sb")
        # PE p-state warm-up: throwaway matmuls keep the PE continuously
        # busy from ~0.4us so it reaches full clock (3us ramp) right as
        # the first projection's data lands -- the real chain then runs
        # at 107ns/matmul instead of 213
        dummy = pers.tile([128, 128], bf16, tag="dmy", name="dmy")
        nc.vector.memset(dummy[:], 0.0)
        for _ in range(32):
            dps = psA.tile([128, 128], f32, tag="psA", name="dps")
            nc.tensor.matmul(dps[:], dummy[:], dummy[:],
                             start=True, stop=True)

        # minimal first-chain working set in pass order: the (qh, kh)
        # half-block feeds passes 1-2, (ql, kl) passes 3-4 -- few issues
        # (the 650ns/DMA queue rate is the startup floor)
        nc.sync.dma_start(ws_sb[:, 0:2048], Wall[:, 0:2048])
        nc.sync.dma_start(xh_sb[:], xh[0])
        nc.sync.dma_start(ws_sb[:, 2048:4096], Wall[:, 2048:4096])
        nc.sync.dma_start(xl_sb[:], xl[0])
        nc.sync.dma_start(bq_sb[:], BQ[:])
        nc.sync.dma_start(bk_sb[:], BK[:])
        xtiles[0] = [xx[:, g * 1024:(g + 1) * 1024].rearrange(
            "p (i t) -> p i t", i=2)
            for xx in (xh_sb, xl_sb) for g in range(NG)]
        nc.vector.memset(vaug4[:, :, :, 64:65], 1.0)

        # pair-1 weights ahead of the remaps (its chain follows at once)
        nc.sync.dma_start(ws_sb[:, 4096:8192], Wall[:, 4096:8192])

        # chunk-0 pair-0 projection only -- the first attention column
        # starts right behind it; the remaining loads are spread as fills
        # in need order so no critical remap queues behind bulk bytes
        proj_qk(0, 0, interleave=True)

        # pair-2..3 weights behind the first remaps, ahead of their chains
        # in column (0,0)'s later fills
        for c0 in range(8192, 16384, 2048):
            nc.sync.dma_start(ws_sb[:, c0:c0 + 2048], Wall[:, c0:c0 + 2048])

        xh1_sb = xsp.tile([128, NG * 1024], f8, tag="xh", name="xh_sb")
        xl1_sb = xsp.tile([128, NG * 1024], f8, tag="xl", name="xl_sb")
        xtiles[1] = [xx[:, g * 1024:(g + 1) * 1024].rearrange(
            "p (i t) -> p i t", i=2)
            for xx in (xh1_sb, xl1_sb) for g in range(NG)]

        def wall_piece(c0):
            nc.sync.dma_start(ws_sb[:, c0:c0 + 2048], Wall[:, c0:c0 + 2048])

        def bv_load():
            nc.sync.dma_start(bv_row[:], BV[:])
            nc.gpsimd.partition_broadcast(bv_bc[:], bv_row[:])

        bulk = (
            [lambda c0=c0: wall_piece(c0)
             for c0 in range(16384, 24576, 2048)] +      # Wv
            [bv_load] +
            [lambda c0=c0: nc.sync.dma_start(
                xh1_sb[:, c0:c0 + 2048], xh[1][:, c0:c0 + 2048])
             for c0 in (0, 2048)] +
            [lambda c0=c0: nc.sync.dma_start(
                xl1_sb[:, c0:c0 + 2048], xl[1][:, c0:c0 + 2048])
             for c0 in (0, 2048)] +
            [lambda t=t: nc.sync.dma_start(
                wo_sb[:, t * D:(t + 1) * D],
                Wo[128 * t:128 * (t + 1), :])
             for t in range(NPAIR)])

        # ---- deadline-packed fill assignment over the k-tile slots ----
        # each non-tail column also exposes its 4 PV-drain positions as
        # fill slots (the drain interleaves next(fill) before each PV)
        last = NCH - 1
        slots = []           # (ch, p, s) in emission order
        slot_of = {}
        for ch in range(NCH):
            for p in range(NPAIR):
                ns = 4 * ch + 4
                if not (ch == last and p == NPAIR - 1):
                    ns += 4
                for s in range(ns):
                    slot_of[(ch, p, s)] = len(slots)
                    slots.append((ch, p, s))
        nslots = len(slots)
        chunk_start = {ch: slot_of[(ch, 0, 0)] for ch in range(NCH)}
        chunk_end = {ch: slot_of[(ch, NPAIR - 1, 4 * ch + 3)]
                     for ch in range(NCH)}

        nsbcs = {}
        items = []  # (deadline, avail, fn, label)
        # chunk 0: V weights + deferred setup DMAs first (they jump no
        # queue), then the remaining q/k chains, V chains at the drains
        # chunk 0: pinned layout -- loads stream in need order, chains one
        # column ahead, V chains just before their carried-PV replays
        pinned = {
            (0, 0, 0): ("qk(0,1)", lambda: proj_qk(0, 1)),
            (0, 0, 1): ("qk(0,2)", lambda: proj_qk(0, 2)),
            (0, 0, 2): ("qk(0,3)", lambda: proj_qk(0, 3)),
            (0, 0, 3): ("wva", bulk[0]),
            (0, 0, 4): ("wvb", bulk[1]),
            (0, 0, 5): ("wvc", bulk[2]),
            (0, 0, 6): ("wvd", bulk[3]),
            (0, 0, 7): ("bv", bulk[4]),
            (0, 1, 0): ("v(0,0)", lambda: proj_v(0, 0)),
            (0, 1, 1): ("x1ha", bulk[5]),
            (0, 1, 2): ("v(0,1)", lambda: proj_v(0, 1)),
            (0, 1, 3): ("x1hb", bulk[6]),
            (0, 1, 4): ("v(0,2)", lambda: proj_v(0, 2)),
            (0, 1, 5): ("x1la", bulk[7]),
            (0, 1, 6): ("v(0,3)", lambda: proj_v(0, 3)),
            (0, 1, 7): ("x1lb", bulk[8]),
            (0, 2, 0): ("woa", bulk[9]),
            (0, 2, 1): ("wob", bulk[10]),
            (0, 2, 2): ("woc", bulk[11]),
            (0, 2, 3): ("wod", bulk[12]),
        }
        for key, (lb, fn) in pinned.items():
            s = slot_of[key]
            items.append((s, s, fn, lb))
        # chunks 1+: q/k chains land one column ahead of use; V chains and
        # the p>=1 q/k chains ride the target chunk's own ACT slack
        for ch in range(1, NCH):
            av = chunk_start[ch - 1]
            # chain->evac->remap->sem is ~5.5us: give 6 slots of margin
            # (more in ch1 whose slots are shorter)
            mg = 10 if ch == 1 else 6
            items.append((slot_of[(ch, 0, 0)] - mg, av,
                          lambda ch=ch: proj_qk(ch, 0), f"qk({ch},0)"))
            # chunk-3 projections pull forward into chunk 2's PE slack so
            # their DVE evacs never queue ahead of the normalizes that the
            # PSUM-pool rotation waits on
            for p in range(1, NPAIR):
                if ch == 3:
                    dl = slot_of[(2, 3, 4 * p)] if p < 3 \
                        else slot_of[(3, 0, 2)]
                    av2 = chunk_start[2]
                else:
                    dl = slot_of[(ch, p, 0)] - (10 if ch == 1 else 6)
                    av2 = chunk_start[ch] if ch == 2 else av
                items.append((dl, av2, lambda ch=ch, p=p: proj_qk(ch, p),
                              f"qk({ch},{p})"))
            for tj in range(4 * ch, 4 * (ch + 1)):
                # the PV group needing this k-tile replays in column
                # (ch, 1) at k-tile (tj - 4ch) + 1
                if ch == 3:
                    dl = slot_of[(3, 0, 8 + 2 * (tj - 12))]
                    av2 = chunk_start[2]
                else:
                    dl = slot_of[(ch, 1, tj - 4 * ch + 1)] - 1
                    av2 = chunk_start[ch] if ch == 2 else av
                items.append((dl, av2, lambda ch=ch, tj=tj: proj_v(ch, tj),
                              f"v({ch},{tj})"))
        for ch in range(NCH - 1):
            # chunk transposes fire after the carried normalize of the
            # chunk's last pair, which replays at k-tile 5 of the next
            # chunk's first column
            av = slot_of[(ch + 1, 0, 6)]
            items.append((av + 1, av,
                          lambda ch=ch: chunk_transpose(
                              ch, nsbcs[ch], 0, NPAIR), f"T({ch})"))
        # output tiles spread over late-chunk columns (clustering them at
        # the end starves the PE mid-chunk and jams the DMA queue); the
        # chunk-2 tiles ride the ACT-paced slack of the last two columns
        for tj in range(4):
            av = slot_of[(1, 1, 0)]
            items.append((slot_of[(2, tj, 10)], av,
                          lambda tj=tj: outproj_tile(tj), f"op({tj})"))
        for tj in range(4, 8):
            av = slot_of[(2, 1, 0)]
            items.append((slot_of[(3, tj - 4, 10)], av,
                          lambda tj=tj: outproj_tile(tj), f"op({tj})"))
        for tj, key in ((8, (3, 2, 10)), (9, (3, 2, 13)),
                        (10, (3, 3, 7)), (11, (3, 3, 10))):
            items.append((slot_of[key], slot_of[key],
                          lambda tj=tj: outproj_tile(tj), f"op({tj})"))
        # the last chunk's pairs-0..2 transpose fires right after the
        # carried normalize of (3,2) lands, clearing the tail's DMA path
        items.append((slot_of[(3, 3, 6)], slot_of[(3, 3, 6)],
                      lambda: chunk_transpose(3, nsbcs[3], 0, NPAIR - 1),
                      "T3p012"))

        assigned = {}
        labels = {}
        for dl, av, fn, lb in sorted(items, key=lambda it: -it[0]):
            s = min(dl, nslots - 1)
            while s >= av and s in assigned:
                s -= 1
            if s < av:
                s = av
                while s in assigned:
                    s += 1
            assigned[s] = fn
            labels[s] = lb
        import os
        if os.environ.get("KERNEL_DEBUG_SLOTS"):
            for s in sorted(labels):
                print(f"slot {s:3d} {slots[s]} <- {labels[s]}")

        fills = {}
        for s, fn in assigned.items():
            fills.setdefault(s, []).append(fn)

        def _slotfill(seq):
            for fl in seq:
                yield (lambda fl=fl: [f() for f in fl]) if fl else None
            while True:
                yield None

        carry = None
        for ch in range(NCH):
            nsbc = nrm.tile([128, NPAIR, 4, 2, 64], bf16, tag="nsbc",
                            name="nsbc")
            nsbcs[ch] = nsbc
            if 0 < ch and ch + 1 < NCH:
                emit_xdma(ch + 1)
            for p in range(NPAIR):
                ns = 4 * ch + 4
                if not (ch == last and p == NPAIR - 1):
                    ns += 4
                seq = []
                for s in range(ns):
                    fl = fills.get(slot_of[(ch, p, s)], [])
                    seq.append(fl)
                if ch == 0:
                    # carry columns pull two fill slots per k-tile
                    seq = [seq[2 * j] + seq[2 * j + 1] for j in range(4)]
                it = _slotfill(seq)
                tailjobs = None
                if ch == last and p == NPAIR - 1:
                    tailjobs = [
                        lambda tj=tj: outproj_tile(tj, act_evac=True)
                        for tj in range(4 * last, 4 * last + 4)]
                if ch == last and p == NPAIR - 1:
                    attn_column(p, ch, it, nsbc, tailjobs, carry_in=carry)
                    carry = None
                else:
                    carry = attn_column(p, ch, it, nsbc, carry_in=carry,
                                        carry_out=True,
                                        fill_first=(ch == 0))
    nc.compile()
    return nc


class _Runner:
    def __init__(self, nc):
        import jax
        from jax.sharding import Mesh, PartitionSpec, NamedSharding
        from jax.experimental.shard_map import shard_map
        import concourse.mybir as mybir
        from concourse.bass2jax import (_bass_exec_p, partition_id_tensor,
                                        install_neuronx_cc_hook)
        install_neuronx_cc_hook()
        self.jax = jax
        part = nc.partition_id_tensor.name if nc.partition_id_tensor else None
        in_names, out_names, out_avals = [], [], []
        for alloc in nc.m.functions[0].allocations:
            if not isinstance(alloc, mybir.MemoryLocationSet):
                continue
            name = alloc.memorylocations[0].name
            if alloc.kind == "ExternalInput":
                if name != part:
                    in_names.append(name)
            elif alloc.kind == "ExternalOutput":
                out_names.append(name)
                out_avals.append(jax.core.ShapedArray(
                    tuple(alloc.tensor_shape), mybir.dt.np(alloc.dtype)))
        self.in_names, self.out_names, self.out_avals = in_names, out_names, out_avals
        all_in = list(in_names) + list(out_names) + ([part] if part else [])

        def _body(*args):
            ops = list(args)
            if part:
                ops.append(partition_id_tensor())
            return tuple(_bass_exec_p.bind(
                *ops, out_avals=tuple(out_avals), in_names=tuple(all_in),
                out_names=tuple(out_names), lowering_input_output_aliases=(),
                sim_require_finite=True, sim_require_nnan=True, nc=nc))

        devices = jax.devices()[:NCORES]
        mesh = Mesh(np.asarray(devices), ("core",))
        nin = len(in_names) + len(out_names)
        self.fn = jax.jit(
            shard_map(_body, mesh=mesh,
                      in_specs=(PartitionSpec("core"),) * nin,
                      out_specs=(PartitionSpec("core"),) * len(out_names),
                      check_rep=False),
            keep_unused=True)
        self.sharding = NamedSharding(mesh, PartitionSpec("core"))

    def put_inputs(self, in_maps):
        args = []
        for name in self.in_names:
            cat = np.concatenate([np.asarray(m[name]) for m in in_maps], axis=0)
            args.append(self.jax.device_put(cat, self.sharding))
        for av in self.out_avals:
            z = np.zeros((NCORES * av.shape[0], *av.shape[1:]), av.dtype)
            args.append(self.jax.device_put(z, self.sharding))
        return args

    def run_np(self, args):
        outs = self.fn(*args)
        return [
            {n: np.asarray(outs[i]).reshape(NCORES, *self.out_avals[i].shape)[c]
             for i, n in enumerate(self.out_names)}
            for c in range(NCORES)
        ]


def _get_runner():
    if "r" not in _CACHE:
        nc = _build()
        _CACHE["nc"] = nc
        _CACHE["r"] = _Runner(nc)
    return _CACHE["r"]


def _rne11(a):
    """Round fp32 to 11 mantissa bits, round-to-nearest-even (= hw fp32r)."""
    ai = np.ascontiguousarray(a, dtype=np.float32).view(np.uint32).astype(np.uint64)
    lsb = (ai >> 12) & 1
    out = (((ai + 2047 + lsb) >> 12) << 12).astype(np.uint32)
    return out.view(np.float32)


def _q8(a):
    import ml_dtypes
    a = np.clip(np.asarray(a, np.float32), -240.0, 240.0)
    return a.astype(ml_dtypes.float8_e4m3)


def _pack_bias(b):
    # [512] -> [128, NPAIR] with the (j, h, s) psum-partition order,
    # pre-scaled by the q/k fp8 storage scale
    br = (b * QKS).reshape(NPAIR, 2, 2, 32).transpose(2, 1, 3, 0)
    return np.ascontiguousarray(br.reshape(128, NPAIR).astype(np.float32))


def make_in_maps(x, Wqkv, bqkv, Wo, bo=None, mask=None):
    # x repack: [ch, part, g, i, tok], feature = 256g + 128i + part
    xhs, xls = [], []
    for b in range(B):
        xr = np.ascontiguousarray(
            x[b].reshape(NCH, 512, NG, 2, 128).transpose(0, 4, 2, 3, 1))
        h8 = _q8(xr)
        l8 = _q8(xr - h8.astype(np.float32))
        xhs.append(h8.reshape(NCH, 128, NG * 1024))
        xls.append(l8.reshape(NCH, 128, NG * 1024))

    in_maps = []
    for c in range(NCORES):
        b, g = c // 2, c % 2
        sl = slice(g * FPC, (g + 1) * FPC)

        def pack_qk(w):
            # [d, col] -> [part, p, g, i, f]: d = 256g+128i+part, col = 128p+f
            # and f (psum partition) reordered to (j, h, s):
            # feature-within-pair = 64h + 32j + s  ->  f = 64j + 32h + s
            wr = (w * WSC).reshape(NG, 2, 128, NPAIR, 2, 2, 32).transpose(
                2, 3, 0, 1, 5, 4, 6)
            hi = _q8(wr)
            lo = _q8(wr - hi.astype(np.float32))
            return (hi.reshape(128, 4096), lo.reshape(128, 4096))

        def pack_v(w):
            # [d, col] -> [part, g, i, f]: d = 256g+128i+part, col = f(512)
            wr = (w * WSC).reshape(NG, 2, 128, FPC).transpose(2, 0, 1, 3)
            hi = _q8(wr)
            lo = _q8(wr - hi.astype(np.float32))
            return (hi.reshape(128, 4096), lo.reshape(128, 4096))

        qh, ql = pack_qk(Wqkv[:, 0 * D:1 * D][:, sl])
        kh, kl = pack_qk(Wqkv[:, 1 * D:2 * D][:, sl])
        vh, vl = pack_v(Wqkv[:, 2 * D:3 * D][:, sl])
        # pack in load order: per pair the (qh, ql, kh, kl) column
        # quarters, then vh, vl
        wall = np.concatenate(
            [w[:, p * 1024:(p + 1) * 1024]
             for p in range(NPAIR) for w in (qh, kh, ql, kl)] + [vh, vl],
            axis=1)

        import ml_dtypes
        in_maps.append({
            "xh": xhs[b], "xl": xls[b],
            "Wall": np.ascontiguousarray(wall),
            "BQ": _pack_bias(bqkv[0 * D:1 * D][sl]),
            "BK": _pack_bias(bqkv[1 * D:2 * D][sl]),
            "BV": np.ascontiguousarray(bqkv[2 * D:3 * D][sl]),
            "Wo": np.ascontiguousarray(Wo[sl, :]).astype(ml_dtypes.bfloat16),
        })
    return in_maps


def kernel(x, Wqkv, bqkv, Wo, bo, mask=None, **_unused):
    x = np.asarray(x, dtype=np.float32)
    Wqkv = np.asarray(Wqkv, dtype=np.float32)
    bqkv = np.asarray(bqkv, dtype=np.float32)
    Wo = np.asarray(Wo, dtype=np.float32)
    bo = np.asarray(bo, dtype=np.float32)
    in_maps = make_in_maps(x, Wqkv, bqkv, Wo)
    last_err = None
    for _attempt in range(3):
        try:
            r = _get_runner()
            args = r.put_inputs(in_maps)
            res = r.run_np(args)
            break
        except Exception as e:  # transient device wedge: retry fresh
            last_err = e
            _CACHE.clear()
            import time
            time.sleep(5)
    else:
        raise last_err
    out = np.empty((B, T, D), dtype=np.float32)
    for b in range(B):
        out[b] = res[2 * b]["y"] + res[2 * b + 1]["y"] + bo
    return out

